# revision 7
# baseline (speedup 1.0000x reference)
"""Multi-head attention (B=2, S=4096, D=512, H=8) on 8 TRN2 NeuronCores.

Sharding: batch x head-pair (tensor parallel). Core c handles batch
b=c//4 and heads {2p, 2p+1} with p=c%4, over the FULL 4096-token
sequence. Q/K/V/O projections are sliced along the head dimension
(each core projects only its 128 dims), eliminating the redundant
full K/V projection of token-sharding. Each core emits a PARTIAL
output (its heads' contribution through w_o); the host sums the four
partials per batch. V/O biases are folded on the host into a per-core
additive vector (boeff); Q/K biases are added on-device via rank-1
matmuls.

Attention is flash-style with scores kept transposed [tk, tq]. The
two heads' score matmuls (contraction 64) are issued back-to-back at
PE row groups 0/64, so they run CONCURRENTLY in the systolic array
(row tiling) - scores cost half of token-sharding. Softmax skips the
max-subtraction (scores ~ N(0,1)) and the denominator comes from a
ones column appended to V, so softmax is exactly one ACT pass per
score block. The kernel is ACT(exp)-bound: 256 activations of
[128,1024] are the critical path; the schedule keeps ACT fed by
emitting next-group scores before current-group PV and interleaving
all projection work into early-group slack.
"""

import numpy as np
import ml_dtypes

B, S, D = 2, 4096, 512
H, DK = 8, 64
N_CORES = 8
PD = 128  # dims per core (2 heads x 64)
NTQ = 8  # tq tiles of 512
NCH = 32  # tk chunks of 128

_PROGRAM = None


def _build_program():
    from contextlib import ExitStack

    import concourse.mybir as mybir
    import concourse.tile as tile
    from concourse import bacc

    bf = mybir.dt.bfloat16
    f32 = mybir.dt.float32
    Exp = mybir.ActivationFunctionType.Exp

    nc = bacc.Bacc(None)

    qT = nc.declare_dram_parameter("qT", [D, S], bf, isOutput=False)
    kT = nc.declare_dram_parameter("kT", [D, S], bf, isOutput=False)
    vT = nc.declare_dram_parameter("vT", [D, S], bf, isOutput=False)
    wqT = nc.declare_dram_parameter("wqT", [D, PD], bf, isOutput=False)
    wkT = nc.declare_dram_parameter("wkT", [D, PD], bf, isOutput=False)
    wvT = nc.declare_dram_parameter("wvT", [D, PD], bf, isOutput=False)
    woT = nc.declare_dram_parameter("woT", [PD, D], bf, isOutput=False)
    bq = nc.declare_dram_parameter("bq", [1, PD], bf, isOutput=False)
    bk = nc.declare_dram_parameter("bk", [1, PD], bf, isOutput=False)
    boeff = nc.declare_dram_parameter("boeff", [1, D], bf, isOutput=False)
    out_p = nc.declare_dram_parameter("out", [S, D], f32, isOutput=True)
    # DRAM scratch rows for softmax denominator / reciprocal broadcasting
    rden = nc.dram_tensor("rden", [NTQ * 2, 512], f32)
    rrec = nc.dram_tensor("rrec", [NTQ * 2, 512], f32)

    with tile.TileContext(nc) as tc, ExitStack() as ctx:
        wpool = ctx.enter_context(tc.tile_pool(name="w", bufs=1))
        kstream = ctx.enter_context(tc.tile_pool(name="kstream", bufs=2))
        qstream = ctx.enter_context(tc.tile_pool(name="qstream", bufs=2))
        vstream = ctx.enter_context(tc.tile_pool(name="vstream", bufs=2))
        khpool = ctx.enter_context(tc.tile_pool(name="kh", bufs=1))
        qhpool = ctx.enter_context(tc.tile_pool(name="qh", bufs=1))
        vstore = ctx.enter_context(tc.tile_pool(name="vstore", bufs=33))
        ptpool = ctx.enter_context(tc.tile_pool(name="pt", bufs=3))
        pvsbp = ctx.enter_context(tc.tile_pool(name="pvsb", bufs=4))
        opool = ctx.enter_context(tc.tile_pool(name="o", bufs=2))
        wsp = ctx.enter_context(tc.tile_pool(name="ws", bufs=6))
        ostage = ctx.enter_context(tc.tile_pool(name="ostage", bufs=2))
        scorep = ctx.enter_context(tc.tile_pool(name="scorep", bufs=2, space="PSUM"))
        pvp = ctx.enter_context(tc.tile_pool(name="pvp", bufs=2, space="PSUM"))
        projp = ctx.enter_context(tc.tile_pool(name="projp", bufs=2, space="PSUM"))

        dma = nc.sync.dma_start
        MM = nc.tensor.matmul

        # ---- constants ----
        ones1 = wpool.tile([1, D], bf, tag="ones", name="ones1")
        nc.vector.memset(ones1[:], 1.0)
        # warm the ACT table (exp) during the DMA-heavy prefix
        wrm = wsp.tile([1, 16], f32, tag="sp", name="warm")
        nc.vector.memset(wrm[:], 0.0)
        wrm2 = wsp.tile([1, 16], f32, tag="sp2", name="warm2")
        nc.scalar.activation(out=wrm2[:], in_=wrm[:], func=Exp, scale=1.0)

        def wtiles(param, tagp):
            t = wpool.tile([128, 4, PD], bf, tag=tagp, name=tagp)
            dma(out=t[:], in_=param[:].rearrange("(c p) d -> p c d", p=128))
            return t

        wq_t = wtiles(wqT, "wq")
        wk_t = wtiles(wkT, "wk")
        wv_t = wtiles(wvT, "wv")
        # [64, 2, D]: wo_t[:, h, :] puts both heads' w_o rows at base
        # partition 0, so out-proj accumulation MMs share row group 0
        # (concurrent row-group accumulation into one PSUM bank races).
        wo_t = wpool.tile([DK, 2, D], bf, tag="wo", name="wo_t")
        dma(out=wo_t[:], in_=woT[:].rearrange("(h p) d -> p h d", p=DK))
        bq_t = wpool.tile([1, PD], bf, tag="bq", name="bq_t")
        dma(out=bq_t[:], in_=bq[:])
        bk_t = wpool.tile([1, PD], bf, tag="bk", name="bk_t")
        dma(out=bk_t[:], in_=bk[:])
        boeff_t = wpool.tile([1, D], bf, tag="boeff", name="boeff_t")
        dma(out=boeff_t[:], in_=boeff[:])

        khT = khpool.tile([PD, S], bf, tag="khT", name="khT")
        qhT = qhpool.tile([PD, S], bf, tag="qhT", name="qhT")
        v_store = [None] * NCH  # [128 tok, 2 heads, DK+1]; col 64 = ones

        kraw_t = {}
        qraw_t = {}
        vraw_t = {}

        def dma_kraw(t):
            kr = kstream.tile([128, 4, 512], bf, tag="kraw", name="kraw")
            dma(
                out=kr[:],
                in_=kT[:, t * 512 : (t + 1) * 512].rearrange(
                    "(c p) t -> p c t", p=128
                ),
            )
            kraw_t[t] = kr

        def dma_qraw(t):
            qr = qstream.tile([128, 4, 512], bf, tag="qraw", name="qraw")
            dma(
                out=qr[:],
                in_=qT[:, t * 512 : (t + 1) * 512].rearrange(
                    "(c p) t -> p c t", p=128
                ),
            )
            qraw_t[t] = qr

        def dma_vraw(r):
            vr = vstream.tile([128, 4, 512], bf, tag="vraw", name="vraw")
            dma(
                out=vr[:],
                in_=vT[:, r * 512 : (r + 1) * 512].rearrange(
                    "(c p) t -> p c t", p=128
                ),
            )
            vraw_t[r] = vr

        def proj_qk(raw, w_t, b_t, dst, t):
            """Project K or Q for token tile t -> dst[:, t*512:(t+1)*512]."""
            ps = projp.tile([128, 512], f32, tag="proj", name="proj_ps")
            for kk in range(4):
                MM(
                    ps[:],
                    w_t[:, kk, :],
                    raw[:, kk, :],
                    start=(kk == 0),
                    stop=False,
                    skip_group_check=True,
                )
            MM(
                ps[:],
                b_t[:],
                ones1[0:1, 0:512],
                start=False,
                stop=True,
                skip_group_check=True,
            )
            nc.vector.tensor_copy(out=dst[:, t * 512 : (t + 1) * 512], in_=ps[:])

        def proj_v_sub(r, sub):
            """Project V tokens (4r+sub)*128.. into v_store[4r+sub].

            Fresh PSUM tile per sub-chunk: sharing one bank across
            sub-chunks makes the DVE copy of chunk n concurrent with PE
            writes of chunk n+1 in the same bank (fatal PSUM collision).
            """
            j = 4 * r + sub
            ps = projp.tile([128, 512], f32, tag="proj", name="vps")
            for kk in range(4):
                MM(
                    ps[:, 0:128],
                    vraw_t[r][:, kk, sub * 128 : (sub + 1) * 128],
                    wv_t[:, kk, :],
                    start=(kk == 0),
                    stop=(kk == 3),
                    skip_group_check=True,
                )
            vs = vstore.tile([128, 2, DK + 1], bf, tag="vs", name="vs")
            v_store[j] = vs
            nc.vector.memset(vs[:, :, DK : DK + 1], 1.0)
            nc.vector.tensor_copy(
                out=vs[:, :, 0:DK],
                in_=ps[:, 0:128].rearrange("p (h c) -> p h c", c=DK),
            )

        def emit_scores(tqt, j):
            sc = scorep.tile([128, 1024], f32, tag="sc", name="sc")
            for h in range(2):
                pb = h * 64
                MM(
                    sc[:, h * 512 : (h + 1) * 512],
                    khT[pb : pb + 64, j * 128 : (j + 1) * 128],
                    qhT[pb : pb + 64, tqt * 512 : (tqt + 1) * 512],
                    start=True,
                    stop=True,
                    skip_group_check=True,
                )
            return sc

        # ---- closure schedule: group index -> list of closures ----
        extra = {}

        def add(g, fn):
            extra.setdefault(g, []).append(fn)

        # K tiles 1-7: dma 4 groups ahead of the matmuls
        for t in range(1, 8):
            add(4 * t - 4, lambda t=t: dma_kraw(t))
            add(
                4 * t - 2,
                lambda t=t: proj_qk(kraw_t[t], wk_t, bk_t, khT, t),
            )
        # V raw streams r=2..7 (r=0,1 in prefix); sub-closure for chunk j
        # runs at group j-4
        for r in range(2, 8):
            add(4 * r - 8, lambda r=r: dma_vraw(r))
        for j in range(4, NCH):
            add(j - 4, lambda r=j // 4, s=j % 4: proj_v_sub(r, s))
        # Q tiles 1-7 projected near the end of the previous tq tile
        for T in range(1, 8):
            add((T - 1) * 32 + 24, lambda T=T: dma_qraw(T))
            add(
                (T - 1) * 32 + 26,
                lambda T=T: proj_qk(qraw_t[T], wq_t, bq_t, qhT, T),
            )

        # ---- per-tq-tile normalization + out-projection closures ----
        def make_close_steps(tqt, pvsb, o):
            steps = []
            for h in range(2):
                i = tqt * 2 + h

                def s1(i=i, pv1=pvsb[h]):
                    dma(out=rden[i : i + 1, :], in_=pv1[64:65, :])
                    sp = wsp.tile([64, 8], f32, tag="sp", name="sp")
                    dma(out=sp[:], in_=rden[i].rearrange("(p e) -> p e", p=64))
                    sp2 = wsp.tile([64, 8], f32, tag="sp2", name="sp2")
                    nc.vector.reciprocal(out=sp2[:], in_=sp[:])
                    dma(out=rrec[i].rearrange("(p e) -> p e", p=64), in_=sp2[:])

                steps.append(s1)
            for h in range(2):
                i = tqt * 2 + h

                def s2(i=i, h=h, pv1=pvsb[h]):
                    w = wsp.tile([64, 512], f32, tag="ws", name="wst")
                    dma(out=w[:], in_=rrec[i : i + 1, :].partition_broadcast(64))
                    nc.vector.tensor_mul(
                        out=o[h][:],
                        in0=pv1[0:64, :],
                        in1=w[:],
                    )

                steps.append(s2)
            for tt in range(4):

                def s3(tt=tt):
                    pa = projp.tile([128, 512], f32, tag="proj", name="out_pa")
                    for h in range(2):
                        MM(
                            pa[:],
                            o[h][:, tt * 128 : (tt + 1) * 128],
                            wo_t[:, h, :],
                            start=(h == 0),
                            stop=False,
                            skip_group_check=True,
                        )
                    MM(
                        pa[:],
                        ones1[0:1, 0:128],
                        boeff_t[:],
                        start=False,
                        stop=True,
                        skip_group_check=True,
                    )
                    ot = ostage.tile([128, 512], f32, tag="ot", name="ot")
                    nc.vector.tensor_copy(out=ot[:], in_=pa[:])
                    dma(
                        out=out_p[
                            tqt * 512 + tt * 128 : tqt * 512 + (tt + 1) * 128, :
                        ],
                        in_=ot[:],
                    )

                steps.append(s3)
            return steps

        CLOSE_SLOTS = (1, 2, 4, 6, 9, 13, 17, 21)
        pend = {}

        # ---- prefix ----
        dma_kraw(0)
        dma_qraw(0)
        dma_vraw(0)
        dma_vraw(1)
        proj_qk(kraw_t[0], wk_t, bk_t, khT, 0)
        proj_qk(qraw_t[0], wq_t, bq_t, qhT, 0)
        for sub in range(4):
            proj_v_sub(0, sub)

        # ---- main attention loop ----
        sc_next = emit_scores(0, 0)
        pv_tiles = None
        for g in range(NTQ * NCH):
            tqt, j = divmod(g, NCH)
            if j == 0:
                pv_tiles = [
                    pvp.tile([DK + 1, 512], f32, tag="pv", name=f"pv{_h}")
                    for _h in range(2)
                ]
            sc = sc_next
            pt = ptpool.tile([128, 1024], bf, tag="pt", name="pt")
            nc.scalar.activation(out=pt[:], in_=sc[:], func=Exp, scale=0.125)
            # interleaved work (projections, previous tile's normalization)
            for fn in extra.get(g, ()):
                fn()
            if tqt >= 1 and j in CLOSE_SLOTS and (tqt - 1) in pend:
                pend[tqt - 1][CLOSE_SLOTS.index(j)]()
            # next group's scores ahead of this group's PV
            if g + 1 < NTQ * NCH:
                ntqt, nj = divmod(g + 1, NCH)
                sc_next = emit_scores(ntqt, nj)
            for h in range(2):
                MM(
                    pv_tiles[h][:],
                    v_store[j][:, h, :],
                    pt[:, h * 512 : (h + 1) * 512],
                    start=(j == 0),
                    stop=(j == NCH - 1),
                    skip_group_check=True,
                )
            if j == NCH - 1:
                pvsb = []
                for h in range(2):
                    t = pvsbp.tile([DK + 1, 512], f32, tag="pvsb", name="pvsb")
                    nc.vector.tensor_copy(out=t[:], in_=pv_tiles[h][:])
                    pvsb.append(t)
                o = [
                    opool.tile([DK, 512], bf, tag="oh", name="oh")
                    for _h in range(2)
                ]
                pend[tqt] = make_close_steps(tqt, pvsb, o)

        # ---- tail: last tq tile's normalization + out-projection ----
        for fn in pend[NTQ - 1]:
            fn()

    if not nc.is_finalized():
        nc.finalize()
    return nc


def _get_program():
    global _PROGRAM
    if _PROGRAM is None:
        _PROGRAM = _build_program()
    return _PROGRAM


def _prep_inputs(q, k, v, w_q, b_q, w_k, b_k, w_v, b_v, w_o, b_o):
    bf16 = ml_dtypes.bfloat16
    q = np.asarray(q, dtype=np.float32)
    k = np.asarray(k, dtype=np.float32)
    v = np.asarray(v, dtype=np.float32)
    w_q = np.asarray(w_q, np.float32)
    w_k = np.asarray(w_k, np.float32)
    w_v = np.asarray(w_v, np.float32)
    w_o = np.asarray(w_o, np.float32)
    b_q = np.asarray(b_q, np.float32)
    b_k = np.asarray(b_k, np.float32)
    b_v = np.asarray(b_v, np.float32)
    b_o = np.asarray(b_o, np.float32)

    qT = [np.ascontiguousarray(q[b].T).astype(bf16) for b in range(B)]
    kTb = [np.ascontiguousarray(k[b].T).astype(bf16) for b in range(B)]
    vTb = [np.ascontiguousarray(v[b].T).astype(bf16) for b in range(B)]
    wqT = np.ascontiguousarray(w_q.T).astype(bf16)  # [D_in, D_out]
    wkT = np.ascontiguousarray(w_k.T).astype(bf16)
    wvT = np.ascontiguousarray(w_v.T).astype(bf16)
    woT = np.ascontiguousarray(w_o.T)  # [D_in(head dims), D_out] f32

    in_maps = []
    for c in range(N_CORES):
        b, p = divmod(c, 4)
        ds = slice(p * PD, (p + 1) * PD)
        be = b_v[ds] @ woT[ds, :] + (b_o if p == 0 else 0.0)
        in_maps.append(
            {
                "qT": qT[b],
                "kT": kTb[b],
                "vT": vTb[b],
                "wqT": np.ascontiguousarray(wqT[:, ds]),
                "wkT": np.ascontiguousarray(wkT[:, ds]),
                "wvT": np.ascontiguousarray(wvT[:, ds]),
                "woT": np.ascontiguousarray(woT[ds, :]).astype(bf16),
                "bq": b_q[ds].reshape(1, PD).astype(bf16),
                "bk": b_k[ds].reshape(1, PD).astype(bf16),
                "boeff": be.reshape(1, D).astype(bf16),
            }
        )
    return in_maps


def run_cores(in_maps, trace=False, **kw):
    """Compile+run the SPMD program; returns BassKernelResults."""
    from concourse.bass_utils import run_bass_kernel_spmd

    nc = _get_program()
    return run_bass_kernel_spmd(nc, in_maps, list(range(N_CORES)), trace=trace, **kw)


def combine_outputs(res):
    """Sum the per-core partial outputs into the full [B, S, D] result."""
    out = np.zeros((B, S, D), np.float32)
    for c in range(N_CORES):
        b = c // 4
        out[b] += res.results[c]["out"]
    return out


def kernel(q, k, v, w_q, b_q, w_k, b_k, w_v, b_v, w_o, b_o):
    in_maps = _prep_inputs(q, k, v, w_q, b_q, w_k, b_k, w_v, b_v, w_o, b_o)
    res = run_cores(in_maps)
    return combine_outputs(res)
